# revision 34
# baseline (speedup 1.0000x reference)
"""GroupLinear (soft MoE routing) Trainium2 Bass kernel.

Computes out[b,o] = sum_j g[b,j] * (x[b,:] @ W[j,:,:])[o] + (g @ bias_p)[b,o]
for B=16384, G=16, DIN=DOUT=512, fp32.

Sharding: data-parallel over batch across 8 NeuronCores (2048 rows/core);
weight + bias replicated.

Per-core schedule (PE-roofline oriented):
  - W resident in SBUF (16 MB), DMA'd directly as float32r (4-byte fp32
    layout; PE reduced-precision mode runs 1 cycle/row at N=512).
  - batch tiles [128, 512] processed in 2 phases of 8 tiles with the
    GROUP loop outermost inside a phase: per (j, bt) four K=128 matmuls
    accumulate Y = x_bt @ W_j in a PSUM bank. This keeps per-W-slice
    compute (~7.5us) far above the W DMA arrival rate (~2.8us/slice), so
    the 16 MB weight load never stalls the PE after the first slice.
  - x transposed on PE via identity (4x [128,128] per tile), PSUM->SBUF
    rounding copies on ScalarE.
  - bias folded in as chain seed: yb = gT @ bias_p (K=16 matmul) at j=0,
    acc = yb + g0*Y0 on VectorE.
  - per (j, bt): Y scaled by g[:,bt][:,j] (per-partition scalar) on
    ScalarE (6 of 8 tiles) or VectorE (2 of 8); group-accumulated into
    acc[bt] on VectorE (5 of 8 tiles) or GpSimdE (3 of 8).
"""

import numpy as np

import concourse.bass as bass
import concourse.tile as tile
from concourse import bacc, mybir
from concourse.bass_utils import run_bass_kernel_spmd
from concourse.masks import make_identity

B, G, DIN, DOUT = 16384, 16, 512, 512
NCORES = 8
BC = B // NCORES          # rows per core
P = 128                   # partitions
NBT = BC // P             # batch tiles per core (16)
KC = DIN // P             # contraction chunks (4)
PB = 8                    # batch tiles per phase
NPH = NBT // PB           # phases (2)

F32 = mybir.dt.float32
F32R = mybir.dt.float32r

# per phase-slot k (0..PB-1): which engine scales Y and which accumulates.
# GpSimd (slowest) gets the earliest slots so the phase tail drains on the
# fast engines; ScalarE scales the slots whose adds are on DVE/GpSimd.
SCALE_ON_ACT = {2, 3, 4, 5, 6, 7}   # else VectorE
CHAIN_ON_DVE = {3, 4, 5, 6, 7}      # else GpSimdE


def _emit(nc, tc, out_ap, x_ap, g_ap, w_ap, bias_ap, ctx):
    const_pool = ctx.enter_context(tc.tile_pool(name="const", bufs=1))
    wpool = ctx.enter_context(tc.tile_pool(name="wpool", bufs=1))
    xpool = ctx.enter_context(tc.tile_pool(name="xpool", bufs=6))
    gpool = ctx.enter_context(tc.tile_pool(name="gpool", bufs=2 * PB + 2))
    xtpool = ctx.enter_context(tc.tile_pool(name="xtpool", bufs=PB + 1))
    gtpool = ctx.enter_context(tc.tile_pool(name="gtpool", bufs=PB + 1))
    accpool = ctx.enter_context(tc.tile_pool(name="accpool", bufs=PB + 1))
    tmppool = ctx.enter_context(tc.tile_pool(name="tmppool", bufs=5))
    ps_y = ctx.enter_context(tc.tile_pool(name="ps_y", bufs=4, space="PSUM"))
    ps_yb = ctx.enter_context(tc.tile_pool(name="ps_yb", bufs=2, space="PSUM"))
    ps_t = ctx.enter_context(tc.tile_pool(name="ps_t", bufs=2, space="PSUM"))

    ident = const_pool.tile([P, P], F32, name="ident")
    make_identity(nc, ident)

    # PE warm-up: a few dependency-free fp32 matmuls right at kernel start
    # so the HAM clock gate reaches 8/8 before the real matmul stream begins
    dum = const_pool.tile([P, DOUT], F32, name="dum")
    nc.gpsimd.memset(dum[:], 1.0)
    for wi in range(4):
        wps = ps_t.tile([P, DOUT], F32, tag="tps", name="wps")
        nc.tensor.matmul(wps[:], dum[:, 0:P], dum[:], start=True, stop=True)

    def issue_load(bt):
        xt = xpool.tile([P, DIN], F32, tag="xt", name=f"xt{bt}")
        nc.sync.dma_start(xt[:], x_ap[bt * P:(bt + 1) * P, :])
        gt = gpool.tile([P, G], F32, tag="gt", name=f"gt{bt}")
        nc.sync.dma_start(gt[:], g_ap[bt * P:(bt + 1) * P, :])
        return xt, gt

    def issue_transpose(xt, gt, bt):
        """PE transposes of x (+g); PSUM->SBUF float32r copies on ScalarE."""
        xT = xtpool.tile([P, DIN], F32R, tag="xT", name=f"xT{bt}")
        for ic in range(KC):
            tps = ps_t.tile([P, P], F32, tag="tps", name="tps")
            nc.tensor.transpose(tps[:], xt[:, ic * P:(ic + 1) * P], ident[:])
            nc.scalar.copy(xT[:, ic * P:(ic + 1) * P], tps[:])
        gps = ps_t.tile([G, P], F32, tag="tps", name="gps")
        nc.tensor.transpose(gps[:], gt[:], ident[:])
        gT = gtpool.tile([G, P], F32R, tag="gT", name=f"gT{bt}")
        nc.scalar.copy(gT[:], gps[:])
        return xT, gT

    # startup ordering: x0/g0 + W[0] first so the first transposes and
    # matmuls can begin within a few microseconds, then the remaining
    # phase-A tiles, then the rest of the weight stream.
    loads = {0: issue_load(0)}
    w_sb = wpool.tile([P, G * KC * DOUT], F32R, name="w_sb")

    def issue_w(j):
        for ic in range(KC):
            nc.sync.dma_start(
                w_sb[:, (j * KC + ic) * DOUT:(j * KC + ic + 1) * DOUT],
                w_ap[j, ic * P:(ic + 1) * P, :],
            )

    issue_w(0)
    bias_sb = const_pool.tile([G, DOUT], F32R, name="bias_sb")
    nc.sync.dma_start(bias_sb[:], bias_ap[:, :])
    # interleave the remaining phase-A x/g loads with the early W slices so
    # both streams progress together (j-step i consumes w[i] at ~7.5us/step,
    # so neither stream needs to finish first)
    for bt in range(1, PB):
        loads[bt] = issue_load(bt)
        issue_w(bt)
    for j in range(PB, G):
        issue_w(j)

    trs = {}
    for ph in range(NPH):
        bts = list(range(ph * PB, (ph + 1) * PB))
        for bt in bts:
            if bt not in trs:
                trs[bt] = issue_transpose(*loads[bt], bt)

        accs = {}
        for j in range(G):
            # emit next phase's transposes just before this phase's last
            # group so their PSUM->SBUF copies drain on ScalarE while the
            # final group's matmuls stream, instead of stalling phase start
            if ph + 1 < NPH and j == G - 1:
                for nbt in range((ph + 1) * PB, (ph + 2) * PB):
                    trs[nbt] = issue_transpose(*loads[nbt], nbt)
            for k, bt in enumerate(bts):
                xT, gT = trs[bt]
                gt = loads[bt][1]
                y = ps_y.tile([P, DOUT], F32, tag="y", name="y")
                for ic in range(KC):
                    nc.tensor.matmul(
                        y[:],
                        xT[:, ic * P:(ic + 1) * P],
                        w_sb[:, (j * KC + ic) * DOUT:(j * KC + ic + 1) * DOUT],
                        start=(ic == 0),
                        stop=(ic == KC - 1),
                    )
                tmp = tmppool.tile([P, DOUT], F32, tag="tmp", name=f"tmp{k}")
                if k in SCALE_ON_ACT:
                    nc.scalar.mul(tmp[:], y[:], gt[:, j:j + 1])
                else:
                    nc.vector.tensor_scalar_mul(tmp[:], y[:], gt[:, j:j + 1])

                if j == 0:
                    # bias term seeds the accumulator chain
                    yb = ps_yb.tile([P, DOUT], F32, tag="yb", name=f"yb{bt}")
                    nc.tensor.matmul(yb[:], gT[:], bias_sb[:], start=True, stop=True)
                    acc = accpool.tile([P, DOUT], F32, tag="acc", name=f"acc{bt}")
                    nc.vector.tensor_add(acc[:], yb[:], tmp[:])
                    accs[bt] = acc
                elif k in CHAIN_ON_DVE:
                    nc.vector.tensor_add(accs[bt][:], accs[bt][:], tmp[:])
                else:
                    nc.gpsimd.tensor_add(accs[bt][:], accs[bt][:], tmp[:])

            # prefetch next phase's x/g mid-phase, staggered
            if ph + 1 < NPH and 7 <= j < 7 + PB:
                nxt = (ph + 1) * PB + (j - 7)
                loads[nxt] = issue_load(nxt)

        for bt in bts:
            nc.sync.dma_start(out_ap[bt * P:(bt + 1) * P, :], accs[bt][:])


def _build():
    nc = bacc.Bacc("TRN2", target_bir_lowering=False, debug=False)
    x_ap = nc.dram_tensor("x", [BC, DIN], F32, kind="ExternalInput").ap()
    g_ap = nc.dram_tensor("g", [BC, G], F32, kind="ExternalInput").ap()
    # weight/bias declared float32r (same 4-byte layout as fp32 on the host)
    # so DMA feeds the FP32r matmuls directly with no conversion pass
    w_ap = nc.dram_tensor("weight", [G, DIN, DOUT], F32R, kind="ExternalInput").ap()
    bias_ap = nc.dram_tensor("bias_p", [G, DOUT], F32R, kind="ExternalInput").ap()
    out_ap = nc.dram_tensor("out", [BC, DOUT], F32, kind="ExternalOutput").ap()

    from contextlib import ExitStack

    with tile.TileContext(nc) as tc:
        with ExitStack() as ctx:
            _emit(nc, tc, out_ap, x_ap, g_ap, w_ap, bias_ap, ctx)
    nc.compile()
    return nc


_NC = None
last_result = None


def kernel(x, g, weight, bias_p):
    global _NC, last_result
    if _NC is None:
        _NC = _build()

    x = np.ascontiguousarray(np.asarray(x, dtype=np.float32))
    g = np.ascontiguousarray(np.asarray(g, dtype=np.float32))
    weight = np.ascontiguousarray(np.asarray(weight, dtype=np.float32))
    bias_p = np.ascontiguousarray(np.asarray(bias_p, dtype=np.float32))

    in_maps = [
        {
            "x": x[c * BC:(c + 1) * BC],
            "g": g[c * BC:(c + 1) * BC],
            "weight": weight,
            "bias_p": bias_p,
        }
        for c in range(NCORES)
    ]
    res = run_bass_kernel_spmd(_NC, in_maps, core_ids=list(range(NCORES)))
    last_result = res
    return np.concatenate([r["out"] for r in res.results], axis=0)


# revision 35
# speedup vs baseline: 1.0414x; 1.0414x over previous
"""GroupLinear (soft MoE routing) Trainium2 Bass kernel.

Computes out[b,o] = sum_j g[b,j] * (x[b,:] @ W[j,:,:])[o] + (g @ bias_p)[b,o]
for B=16384, G=16, DIN=DOUT=512, fp32.

Sharding: data-parallel over batch across 8 NeuronCores (2048 rows/core);
weight + bias replicated.

Per-core schedule (PE-roofline oriented):
  - W resident in SBUF (16 MB), DMA'd directly as float32r (4-byte fp32
    layout; PE reduced-precision mode runs 1 cycle/row at N=512).
  - batch tiles [128, 512] processed in 2 phases of 8 tiles with the
    GROUP loop outermost inside a phase: per (j, bt) four K=128 matmuls
    accumulate Y = x_bt @ W_j in a PSUM bank. This keeps per-W-slice
    compute (~7.5us) far above the W DMA arrival rate (~2.8us/slice), so
    the 16 MB weight load never stalls the PE after the first slice.
  - x transposed on PE via identity (4x [128,128] per tile), PSUM->SBUF
    rounding copies on ScalarE.
  - bias folded in as chain seed: yb = gT @ bias_p (K=16 matmul) at j=0,
    acc = yb + g0*Y0 on VectorE.
  - per (j, bt): Y scaled by g[:,bt][:,j] (per-partition scalar) on
    ScalarE (6 of 8 tiles) or VectorE (2 of 8); group-accumulated into
    acc[bt] on VectorE (5 of 8 tiles) or GpSimdE (3 of 8).
"""

import numpy as np

import concourse.bass as bass
import concourse.tile as tile
from concourse import bacc, mybir
from concourse.bass_utils import run_bass_kernel_spmd
from concourse.masks import make_identity

B, G, DIN, DOUT = 16384, 16, 512, 512
NCORES = 8
BC = B // NCORES          # rows per core
P = 128                   # partitions
NBT = BC // P             # batch tiles per core (16)
KC = DIN // P             # contraction chunks (4)
PB = 8                    # batch tiles per phase
NPH = NBT // PB           # phases (2)

F32 = mybir.dt.float32
F32R = mybir.dt.float32r

# per phase-slot k (0..PB-1): which engine scales Y and which accumulates.
# GpSimd (slowest) gets the earliest slots so the phase tail drains on the
# fast engines; ScalarE scales the slots whose adds are on DVE/GpSimd.
SCALE_ON_ACT = {2, 3, 4, 5, 6, 7}   # else VectorE
CHAIN_ON_DVE = {3, 4, 5, 6, 7}      # else GpSimdE


def _emit(nc, tc, out_ap, x_ap, g_ap, w_ap, bias_ap, ctx):
    const_pool = ctx.enter_context(tc.tile_pool(name="const", bufs=1))
    wpool = ctx.enter_context(tc.tile_pool(name="wpool", bufs=1))
    xpool = ctx.enter_context(tc.tile_pool(name="xpool", bufs=6))
    gpool = ctx.enter_context(tc.tile_pool(name="gpool", bufs=2 * PB + 2))
    xtpool = ctx.enter_context(tc.tile_pool(name="xtpool", bufs=PB + 1))
    gtpool = ctx.enter_context(tc.tile_pool(name="gtpool", bufs=PB + 1))
    accpool = ctx.enter_context(tc.tile_pool(name="accpool", bufs=PB + 1))
    tmppool = ctx.enter_context(tc.tile_pool(name="tmppool", bufs=5))
    ps_y = ctx.enter_context(tc.tile_pool(name="ps_y", bufs=4, space="PSUM"))
    ps_yb = ctx.enter_context(tc.tile_pool(name="ps_yb", bufs=2, space="PSUM"))
    ps_t = ctx.enter_context(tc.tile_pool(name="ps_t", bufs=2, space="PSUM"))

    ident = const_pool.tile([P, P], F32, name="ident")
    make_identity(nc, ident)

    # PE warm-up: a few dependency-free fp32 matmuls right at kernel start
    # so the HAM clock gate reaches 8/8 before the real matmul stream begins
    dum = const_pool.tile([P, DOUT], F32, name="dum")
    nc.gpsimd.memset(dum[:], 1.0)
    for wi in range(4):
        wps = ps_t.tile([P, DOUT], F32, tag="tps", name="wps")
        nc.tensor.matmul(wps[:], dum[:, 0:P], dum[:], start=True, stop=True)

    def issue_load(bt):
        xt = xpool.tile([P, DIN], F32, tag="xt", name=f"xt{bt}")
        nc.sync.dma_start(xt[:], x_ap[bt * P:(bt + 1) * P, :])
        gt = gpool.tile([P, G], F32, tag="gt", name=f"gt{bt}")
        nc.sync.dma_start(gt[:], g_ap[bt * P:(bt + 1) * P, :])
        return xt, gt

    def issue_transpose(xt, gt, bt):
        """PE transposes of x (+g); PSUM->SBUF float32r copies on ScalarE."""
        xT = xtpool.tile([P, DIN], F32R, tag="xT", name=f"xT{bt}")
        for ic in range(KC):
            tps = ps_t.tile([P, P], F32, tag="tps", name="tps")
            nc.tensor.transpose(tps[:], xt[:, ic * P:(ic + 1) * P], ident[:])
            nc.scalar.copy(xT[:, ic * P:(ic + 1) * P], tps[:])
        gps = ps_t.tile([G, P], F32, tag="tps", name="gps")
        nc.tensor.transpose(gps[:], gt[:], ident[:])
        gT = gtpool.tile([G, P], F32R, tag="gT", name=f"gT{bt}")
        nc.scalar.copy(gT[:], gps[:])
        return xT, gT

    # startup ordering: x0/g0 + W[0] first so the first transposes and
    # matmuls can begin within a few microseconds, then the remaining
    # phase-A tiles, then the rest of the weight stream.
    loads = {0: issue_load(0)}
    w_sb = wpool.tile([P, G * KC * DOUT], F32R, name="w_sb")

    def issue_w(j):
        for ic in range(KC):
            nc.sync.dma_start(
                w_sb[:, (j * KC + ic) * DOUT:(j * KC + ic + 1) * DOUT],
                w_ap[j, ic * P:(ic + 1) * P, :],
            )

    issue_w(0)
    for bt in range(1, PB):
        loads[bt] = issue_load(bt)
    bias_sb = const_pool.tile([G, DOUT], F32R, name="bias_sb")
    nc.sync.dma_start(bias_sb[:], bias_ap[:, :])
    for j in range(1, G):
        issue_w(j)

    trs = {}
    for ph in range(NPH):
        bts = list(range(ph * PB, (ph + 1) * PB))
        for bt in bts:
            if bt not in trs:
                trs[bt] = issue_transpose(*loads[bt], bt)

        accs = {}
        for j in range(G):
            # emit next phase's transposes just before this phase's last
            # group so their PSUM->SBUF copies drain on ScalarE while the
            # final group's matmuls stream, instead of stalling phase start
            if ph + 1 < NPH and j == G - 1:
                for nbt in range((ph + 1) * PB, (ph + 2) * PB):
                    trs[nbt] = issue_transpose(*loads[nbt], nbt)
            for k, bt in enumerate(bts):
                xT, gT = trs[bt]
                gt = loads[bt][1]
                y = ps_y.tile([P, DOUT], F32, tag="y", name="y")
                for ic in range(KC):
                    nc.tensor.matmul(
                        y[:],
                        xT[:, ic * P:(ic + 1) * P],
                        w_sb[:, (j * KC + ic) * DOUT:(j * KC + ic + 1) * DOUT],
                        start=(ic == 0),
                        stop=(ic == KC - 1),
                    )
                tmp = tmppool.tile([P, DOUT], F32, tag="tmp", name=f"tmp{k}")
                if k in SCALE_ON_ACT:
                    nc.scalar.mul(tmp[:], y[:], gt[:, j:j + 1])
                else:
                    nc.vector.tensor_scalar_mul(tmp[:], y[:], gt[:, j:j + 1])

                if j == 0:
                    # bias term seeds the accumulator chain
                    yb = ps_yb.tile([P, DOUT], F32, tag="yb", name=f"yb{bt}")
                    nc.tensor.matmul(yb[:], gT[:], bias_sb[:], start=True, stop=True)
                    acc = accpool.tile([P, DOUT], F32, tag="acc", name=f"acc{bt}")
                    nc.vector.tensor_add(acc[:], yb[:], tmp[:])
                    accs[bt] = acc
                elif k in CHAIN_ON_DVE:
                    nc.vector.tensor_add(accs[bt][:], accs[bt][:], tmp[:])
                else:
                    nc.gpsimd.tensor_add(accs[bt][:], accs[bt][:], tmp[:])

            # prefetch next phase's x/g mid-phase, staggered
            if ph + 1 < NPH and 7 <= j < 7 + PB:
                nxt = (ph + 1) * PB + (j - 7)
                loads[nxt] = issue_load(nxt)

        for bt in bts:
            nc.sync.dma_start(out_ap[bt * P:(bt + 1) * P, :], accs[bt][:])


def _build():
    nc = bacc.Bacc("TRN2", target_bir_lowering=False, debug=False)
    x_ap = nc.dram_tensor("x", [BC, DIN], F32, kind="ExternalInput").ap()
    g_ap = nc.dram_tensor("g", [BC, G], F32, kind="ExternalInput").ap()
    # weight/bias declared float32r (same 4-byte layout as fp32 on the host)
    # so DMA feeds the FP32r matmuls directly with no conversion pass
    w_ap = nc.dram_tensor("weight", [G, DIN, DOUT], F32R, kind="ExternalInput").ap()
    bias_ap = nc.dram_tensor("bias_p", [G, DOUT], F32R, kind="ExternalInput").ap()
    out_ap = nc.dram_tensor("out", [BC, DOUT], F32, kind="ExternalOutput").ap()

    from contextlib import ExitStack

    with tile.TileContext(nc) as tc:
        with ExitStack() as ctx:
            _emit(nc, tc, out_ap, x_ap, g_ap, w_ap, bias_ap, ctx)
    nc.compile()
    return nc


_NC = None
last_result = None


def kernel(x, g, weight, bias_p):
    global _NC, last_result
    if _NC is None:
        _NC = _build()

    x = np.ascontiguousarray(np.asarray(x, dtype=np.float32))
    g = np.ascontiguousarray(np.asarray(g, dtype=np.float32))
    weight = np.ascontiguousarray(np.asarray(weight, dtype=np.float32))
    bias_p = np.ascontiguousarray(np.asarray(bias_p, dtype=np.float32))

    in_maps = [
        {
            "x": x[c * BC:(c + 1) * BC],
            "g": g[c * BC:(c + 1) * BC],
            "weight": weight,
            "bias_p": bias_p,
        }
        for c in range(NCORES)
    ]
    res = run_bass_kernel_spmd(_NC, in_maps, core_ids=list(range(NCORES)))
    last_result = res
    return np.concatenate([r["out"] for r in res.results], axis=0)
